# revision 20
# baseline (speedup 1.0000x reference)
"""DiceLoss kernel for Trainium2 (8 NeuronCores, pure data parallel).

Problem: softmax over C=19 classes of predict [8, 19, 512, 512], one-hot of
target [8, 512, 512], then per-sample per-class sums
    psum[n,c]  = sum_pix softmax(x)[n,c,pix]
    inter[n,c] = sum_{pix: t=c} softmax(x)[n,c,pix]
    tsum[n,c]  = #{pix: t=c}
and dice = mean_c mean_n (1 - (2*inter+1)/(psum+tsum+1)).

Sharding: one sample per core (batch N=8 across 8 cores). Each core returns
[3*C] partial sums; the tiny final formula runs on host.

Device layout per core: x as [C, 128, 2048] bf16 (pixel-partition,
class-blocked free dim), processed in column chunks of F=512:
  - ScalarE: one Exp activation per chunk over all classes
  - DVE: pairwise-tree class sum -> denominator, reciprocal,
    then per class three fused ops:
      tensor_scalar(is_equal, accum)        -> one-hot mask + tsum column
      tensor_tensor_reduce(E*R, add-accum)  -> probs + psum column (chunk-chained)
      tensor_tensor_reduce(O*P, add-accum)  -> inter column (chunk-chained)
  - TensorE: final [128, 57] x ones matmul to reduce partitions.
Inputs are cast to bf16 on host (halves DMA bytes; fp32 accumulation of the
reductions keeps the loss error ~1e-4).
"""

import numpy as np
import ml_dtypes

N, C, H, W = 8, 19, 512, 512
PIX = H * W  # 262144
P = 128
FTOT = PIX // P  # 2048
F = 512
NCHUNK = FTOT // F
NCORES = 8

_PROG = None


def _build_program():
    from contextlib import ExitStack

    import concourse.bass as bass
    import concourse.tile as tile
    from concourse import mybir

    dt = mybir.dt
    Alu = mybir.AluOpType
    Act = mybir.ActivationFunctionType

    import bass_rust as _br

    class _TC(tile.TileContext):
        # This walrus build rejects instructions carrying more than ONE
        # sync-wait ("Too many sync wait commands", matching bass_rust's
        # inst_waits_full cap), but Tile's wait assignment occasionally
        # stacks 2-3 waits on one instruction and puts one wait per active
        # proc (12 here) on the tail drain. Hoist every excess wait onto an
        # injected same-engine single-wait Drain placed just before.

        def _split_excess_waits(self, insts):
            out = []
            for inst in insts:
                si = inst.sync_info
                if si is not None and len(si.on_wait) > 1:
                    moved = []
                    while len(si.on_wait) > 1:
                        moved.append(si.on_wait.pop())
                    for w in reversed(moved):
                        d = mybir.InstDrain(
                            name=self.nc.get_next_instruction_name(),
                            ins=[],
                            outs=[],
                            bass_is_fusable=False,
                        )
                        d.engine = inst.engine
                        d.sync_info = _br.SyncInfo(on_wait=[w], on_update=[])
                        self.nc.register_instruction(d)
                        out.append(d)
                out.append(inst)
            insts[:] = out

        def _lower_ordered_insts(self, ordered):
            for insts in ordered.values():
                self._split_excess_waits(insts)
            return super()._lower_ordered_insts(ordered)

        def _drain_and_barrier(self, tick_clock, wait_clock):
            from concourse.vector_clock import ScopedClock

            nc = self.nc
            drain_inst = nc.sync.drain()
            wait_clock.add_sem_waits(
                drain_inst.ins, ScopedClock({None: tick_clock.global_clock})
            )
            si = drain_inst.ins.sync_info
            moved = []
            while len(si.on_wait) > 1:
                moved.append(si.on_wait.pop())
            for w in moved:
                d2 = nc.sync.drain()
                d2.ins.sync_info = _br.SyncInfo(on_wait=[w], on_update=[])

            nc.all_engine_barrier()
            assert self.sems is not None
            popped = nc._tile_sem_poison_stack.pop()
            assert popped is self._sem_poison
            nc.clear_and_free_semaphores(list(self.sems.allocated().values()))
            nc.all_engine_barrier()

    nc = bass.Bass(
        "TRN2", target_bir_lowering=False, debug=False, num_devices=NCORES
    )
    x_d = nc.dram_tensor("x", [C, P, FTOT], dt.bfloat16, kind="ExternalInput").ap()
    t_d = nc.dram_tensor("t", [P, FTOT], dt.bfloat16, kind="ExternalInput").ap()
    out_d = nc.dram_tensor("out", [C, 3], dt.float32, kind="ExternalOutput").ap()

    with nc.allow_low_precision("bf16 softmax-stat kernel"), \
            _TC(nc) as tc, ExitStack() as ctx:
        # DMA-written tiles get one slot per chunk: a DMACopy may carry at
        # most ONE sync-wait on TRN2, so slot reuse (which adds WAR/WAW
        # waits onto the DMA) must be avoided for them.
        xp = ctx.enter_context(tc.tile_pool(name="xp", bufs=NCHUNK))
        ep = ctx.enter_context(tc.tile_pool(name="ep", bufs=2))
        tp = ctx.enter_context(tc.tile_pool(name="tp", bufs=NCHUNK))
        dp = ctx.enter_context(tc.tile_pool(name="dp", bufs=2))
        sp = ctx.enter_context(tc.tile_pool(name="sp", bufs=3))
        cp = ctx.enter_context(tc.tile_pool(name="cp", bufs=1))
        pp = ctx.enter_context(tc.tile_pool(name="pp", bufs=1, space="PSUM"))

        # per-class one-hot lhsT columns: block c is a [P, C] matrix whose
        # column c is all-ones -> matmul with rhs [P, F] lands the
        # pixel-partition sums of rhs on PSUM partition c.
        cols = cp.tile([P, C * C], dt.bfloat16)
        nc.vector.memset(cols[:], 0.0)
        for c in range(C):
            nc.vector.memset(cols[:, c * C + c : c * C + c + 1], 1.0)

        ps_acc = pp.tile([C, F], dt.float32)
        in_acc = pp.tile([C, F], dt.float32)
        ts_acc = pp.tile([C, F], dt.float32)

        for j in range(NCHUNK):
            xt = xp.tile([P, C * F], dt.bfloat16, tag="x")
            nc.sync.dma_start(
                out=xt[:].rearrange("p (c f) -> p c f", c=C),
                in_=x_d[:, :, j * F : (j + 1) * F].rearrange("c p f -> p c f"),
            )
            tt = tp.tile([P, F], dt.bfloat16, tag="t")
            nc.sync.dma_start(out=tt[:], in_=t_d[:, j * F : (j + 1) * F])

            et = ep.tile([P, C * F], dt.bfloat16, tag="e")
            nc.scalar.activation(et[:], xt[:], Act.Exp)
            ev = et[:].rearrange("p (c f) -> p c f", c=C)

            # denominator: tree-sum the 19 class blocks (bf16 2x mode),
            # last add in fp32 for reciprocal_approx_fast
            s9 = sp.tile([P, 9 * F], dt.bfloat16, tag="s9")
            s9v = s9[:].rearrange("p (c f) -> p c f", c=9)
            nc.vector.tensor_tensor(
                s9v[:, :, :], ev[:, 0:18:2, :], ev[:, 1:19:2, :], Alu.add
            )
            s4 = sp.tile([P, 4 * F], dt.bfloat16, tag="s4")
            s4v = s4[:].rearrange("p (c f) -> p c f", c=4)
            nc.vector.tensor_tensor(
                s4v[:, :, :], s9v[:, 0:8:2, :], s9v[:, 1:9:2, :], Alu.add
            )
            s2 = sp.tile([P, 2 * F], dt.bfloat16, tag="s2")
            s2v = s2[:].rearrange("p (c f) -> p c f", c=2)
            nc.vector.tensor_tensor(
                s2v[:, :, :], s4v[:, 0:4:2, :], s4v[:, 1:4:2, :], Alu.add
            )
            s1 = sp.tile([P, F], dt.bfloat16, tag="s1")
            nc.vector.tensor_tensor(s1[:], s2v[:, 0, :], s2v[:, 1, :], Alu.add)
            d0 = sp.tile([P, F], dt.bfloat16, tag="d0")
            nc.vector.tensor_tensor(d0[:], s1[:], s9v[:, 8, :], Alu.add)
            dd = sp.tile([P, F], dt.bfloat16, tag="dd")
            nc.vector.tensor_tensor(dd[:], d0[:], ev[:, 18, :], Alu.add)
            rt = dp.tile([P, F], dt.bfloat16, tag="r")
            nc.vector.reciprocal(rt[:], dd[:])

            for c in range(C):
                first = j == 0 and c == 0
                last = j == NCHUNK - 1 and c == C - 1
                lhs = cols[:, c * C : (c + 1) * C]
                o = sp.tile([P, F], dt.bfloat16, tag="o")
                nc.gpsimd.tensor_scalar(o[:], tt[:], float(c), None, Alu.is_equal)
                pc = sp.tile([P, F], dt.bfloat16, tag="pc")
                nc.vector.tensor_tensor(pc[:], ev[:, c, :], rt[:], Alu.mult)
                oc = sp.tile([P, F], dt.bfloat16, tag="oc")
                nc.vector.tensor_tensor(oc[:], o[:], pc[:], Alu.mult)
                nc.tensor.matmul(
                    ps_acc[:], lhsT=lhs, rhs=pc[:], start=first, stop=last
                )
                nc.tensor.matmul(
                    in_acc[:], lhsT=lhs, rhs=oc[:], start=first, stop=last
                )
                nc.tensor.matmul(
                    ts_acc[:], lhsT=lhs, rhs=o[:], start=first, stop=last
                )

        # free-dim reduce of the three PSUM accumulators -> [C, 3] -> DRAM
        ob = cp.tile([C, 3], dt.float32)
        for k, acc in enumerate((ps_acc, in_acc, ts_acc)):
            nc.vector.tensor_reduce(
                out=ob[:, k : k + 1],
                in_=acc[:],
                axis=mybir.AxisListType.X,
                op=Alu.add,
            )
        nc.sync.dma_start(out=out_d[:], in_=ob[:])

    return nc


def _get_program():
    global _PROG
    if _PROG is None:
        _PROG = _build_program()
    return _PROG


def _shard_inputs(predict, target):
    x = np.ascontiguousarray(predict, dtype=np.float32).reshape(N, C, P, FTOT)
    x = x.astype(ml_dtypes.bfloat16)
    t = (
        np.ascontiguousarray(target)
        .reshape(N, P, FTOT)
        .astype(np.float32)
        .astype(ml_dtypes.bfloat16)
    )
    return [{"x": x[i], "t": t[i]} for i in range(N)]


def kernel(predict, target):
    from concourse.bass_utils import run_bass_kernel_spmd

    nc = _get_program()
    in_maps = _shard_inputs(predict, target)
    res = run_bass_kernel_spmd(nc, in_maps, list(range(NCORES)))
    stats = np.stack(
        [np.asarray(res.results[i]["out"], dtype=np.float32).reshape(C, 3) for i in range(NCORES)]
    )
    psum = stats[:, :, 0]
    inter = stats[:, :, 1]
    tsum = stats[:, :, 2]
    top = 2.0 * inter + 1.0
    bot = psum + tsum + 1.0
    per_class = np.mean(1.0 - top / bot, axis=0, dtype=np.float32)
    return np.float32(per_class.sum() / C)


# revision 21
# speedup vs baseline: 4.4931x; 4.4931x over previous
"""DiceLoss kernel for Trainium2 (8 NeuronCores, pure data parallel).

Problem: softmax over C=19 classes of predict [8, 19, 512, 512], one-hot of
target [8, 512, 512], then per-sample per-class sums
    psum[n,c]  = sum_pix softmax(x)[n,c,pix]
    inter[n,c] = sum_{pix: t=c} softmax(x)[n,c,pix]
    tsum[n,c]  = #{pix: t=c}
and dice = mean_c mean_n (1 - (2*inter+1)/(psum+tsum+1)).

Sharding: one sample per core (batch N=8 across 8 cores). Each core returns
[3*C] partial sums; the tiny final formula runs on host.

Device layout per core: x as [C, 128, 2048] bf16 (pixel-partition,
class-blocked free dim), processed in column chunks of F=512:
  - ScalarE: one Exp activation per chunk over all classes
  - DVE: pairwise-tree class sum -> denominator, reciprocal,
    then per class three fused ops:
      tensor_scalar(is_equal, accum)        -> one-hot mask + tsum column
      tensor_tensor_reduce(E*R, add-accum)  -> probs + psum column (chunk-chained)
      tensor_tensor_reduce(O*P, add-accum)  -> inter column (chunk-chained)
  - TensorE: final [128, 57] x ones matmul to reduce partitions.
Inputs are cast to bf16 on host (halves DMA bytes; fp32 accumulation of the
reductions keeps the loss error ~1e-4).
"""

import numpy as np
import ml_dtypes

N, C, H, W = 8, 19, 512, 512
PIX = H * W  # 262144
P = 128
FTOT = PIX // P  # 2048
F = 512
NCHUNK = FTOT // F
NCORES = 8

_PROG = None


def _build_program():
    from contextlib import ExitStack

    import concourse.bass as bass
    import concourse.tile as tile
    from concourse import mybir

    dt = mybir.dt
    Alu = mybir.AluOpType
    Act = mybir.ActivationFunctionType

    import bass_rust as _br

    class _TC(tile.TileContext):
        # This walrus build rejects instructions carrying more than ONE
        # sync-wait ("Too many sync wait commands", matching bass_rust's
        # inst_waits_full cap), but Tile's wait assignment occasionally
        # stacks 2-3 waits on one instruction and puts one wait per active
        # proc (12 here) on the tail drain. Hoist every excess wait onto an
        # injected same-engine single-wait Drain placed just before.

        def _split_excess_waits(self, insts):
            out = []
            for inst in insts:
                si = inst.sync_info
                if si is not None and len(si.on_wait) > 1:
                    moved = []
                    while len(si.on_wait) > 1:
                        moved.append(si.on_wait.pop())
                    for w in reversed(moved):
                        d = mybir.InstDrain(
                            name=self.nc.get_next_instruction_name(),
                            ins=[],
                            outs=[],
                            bass_is_fusable=False,
                        )
                        d.engine = inst.engine
                        d.sync_info = _br.SyncInfo(on_wait=[w], on_update=[])
                        self.nc.register_instruction(d)
                        out.append(d)
                out.append(inst)
            insts[:] = out

        def _lower_ordered_insts(self, ordered):
            for insts in ordered.values():
                self._split_excess_waits(insts)
            return super()._lower_ordered_insts(ordered)

        def _drain_and_barrier(self, tick_clock, wait_clock):
            from concourse.vector_clock import ScopedClock

            nc = self.nc
            drain_inst = nc.sync.drain()
            wait_clock.add_sem_waits(
                drain_inst.ins, ScopedClock({None: tick_clock.global_clock})
            )
            si = drain_inst.ins.sync_info
            moved = []
            while len(si.on_wait) > 1:
                moved.append(si.on_wait.pop())
            for w in moved:
                d2 = nc.sync.drain()
                d2.ins.sync_info = _br.SyncInfo(on_wait=[w], on_update=[])

            nc.all_engine_barrier()
            assert self.sems is not None
            popped = nc._tile_sem_poison_stack.pop()
            assert popped is self._sem_poison
            nc.clear_and_free_semaphores(list(self.sems.allocated().values()))
            nc.all_engine_barrier()

    nc = bass.Bass(
        "TRN2", target_bir_lowering=False, debug=False, num_devices=NCORES
    )
    x_d = nc.dram_tensor("x", [C, P, FTOT], dt.bfloat16, kind="ExternalInput").ap()
    t_d = nc.dram_tensor("t", [P, FTOT], dt.bfloat16, kind="ExternalInput").ap()
    out_d = nc.dram_tensor("out", [C, 3], dt.float32, kind="ExternalOutput").ap()

    with nc.allow_low_precision("bf16 softmax-stat kernel"), \
            _TC(nc) as tc, ExitStack() as ctx:
        # DMA-written tiles get one slot per chunk: a DMACopy may carry at
        # most ONE sync-wait on TRN2, so slot reuse (which adds WAR/WAW
        # waits onto the DMA) must be avoided for them.
        xp = ctx.enter_context(tc.tile_pool(name="xp", bufs=NCHUNK))
        ep = ctx.enter_context(tc.tile_pool(name="ep", bufs=2))
        tp = ctx.enter_context(tc.tile_pool(name="tp", bufs=NCHUNK))
        dp = ctx.enter_context(tc.tile_pool(name="dp", bufs=2))
        sp = ctx.enter_context(tc.tile_pool(name="sp", bufs=3))
        cp = ctx.enter_context(tc.tile_pool(name="cp", bufs=1))
        pp = ctx.enter_context(tc.tile_pool(name="pp", bufs=1, space="PSUM"))

        # per-class one-hot lhsT columns: block c is a [P, C] matrix whose
        # column c is all-ones -> matmul with rhs [P, F] lands the
        # pixel-partition sums of rhs on PSUM partition c.
        cols = cp.tile([P, C * C], dt.bfloat16)
        nc.vector.memset(cols[:], 0.0)
        for c in range(C):
            nc.vector.memset(cols[:, c * C + c : c * C + c + 1], 1.0)

        ps_acc = pp.tile([C, F], dt.float32)
        in_acc = pp.tile([C, F], dt.float32)
        ts_acc = pp.tile([C, F], dt.float32)

        for j in range(NCHUNK):
            xt = xp.tile([P, C * F], dt.bfloat16, tag="x")
            nc.sync.dma_start(
                out=xt[:].rearrange("p (c f) -> p c f", c=C),
                in_=x_d[:, :, j * F : (j + 1) * F].rearrange("c p f -> p c f"),
            )
            tt = tp.tile([P, F], dt.bfloat16, tag="t")
            nc.scalar.dma_start(out=tt[:], in_=t_d[:, j * F : (j + 1) * F])

            et = ep.tile([P, C * F], dt.bfloat16, tag="e")
            nc.scalar.activation(et[:], xt[:], Act.Exp)
            ev = et[:].rearrange("p (c f) -> p c f", c=C)

            # denominator: tree-sum the 19 class blocks (bf16 2x mode),
            # last add in fp32 for reciprocal_approx_fast
            s9 = sp.tile([P, 9 * F], dt.bfloat16, tag="s9")
            s9v = s9[:].rearrange("p (c f) -> p c f", c=9)
            nc.vector.tensor_tensor(
                s9v[:, :, :], ev[:, 0:18:2, :], ev[:, 1:19:2, :], Alu.add
            )
            s4 = sp.tile([P, 4 * F], dt.bfloat16, tag="s4")
            s4v = s4[:].rearrange("p (c f) -> p c f", c=4)
            nc.vector.tensor_tensor(
                s4v[:, :, :], s9v[:, 0:8:2, :], s9v[:, 1:9:2, :], Alu.add
            )
            s2 = sp.tile([P, 2 * F], dt.bfloat16, tag="s2")
            s2v = s2[:].rearrange("p (c f) -> p c f", c=2)
            nc.vector.tensor_tensor(
                s2v[:, :, :], s4v[:, 0:4:2, :], s4v[:, 1:4:2, :], Alu.add
            )
            s1 = sp.tile([P, F], dt.bfloat16, tag="s1")
            nc.vector.tensor_tensor(s1[:], s2v[:, 0, :], s2v[:, 1, :], Alu.add)
            d0 = sp.tile([P, F], dt.bfloat16, tag="d0")
            nc.vector.tensor_tensor(d0[:], s1[:], s9v[:, 8, :], Alu.add)
            dd = sp.tile([P, F], dt.bfloat16, tag="dd")
            nc.vector.tensor_tensor(dd[:], d0[:], ev[:, 18, :], Alu.add)
            rt = dp.tile([P, F], dt.bfloat16, tag="r")
            nc.vector.reciprocal(rt[:], dd[:])

            for c in range(C):
                first = j == 0 and c == 0
                last = j == NCHUNK - 1 and c == C - 1
                lhs = cols[:, c * C : (c + 1) * C]
                o = sp.tile([P, F], dt.bfloat16, tag="o")
                nc.vector.tensor_scalar(o[:], tt[:], float(c), None, Alu.is_equal)
                pc = sp.tile([P, F], dt.bfloat16, tag="pc")
                nc.vector.tensor_tensor(pc[:], ev[:, c, :], rt[:], Alu.mult)
                oc = sp.tile([P, F], dt.bfloat16, tag="oc")
                nc.vector.tensor_tensor(oc[:], o[:], pc[:], Alu.mult)
                nc.tensor.matmul(
                    ps_acc[:], lhsT=lhs, rhs=pc[:], start=first, stop=last
                )
                nc.tensor.matmul(
                    in_acc[:], lhsT=lhs, rhs=oc[:], start=first, stop=last
                )
                nc.tensor.matmul(
                    ts_acc[:], lhsT=lhs, rhs=o[:], start=first, stop=last
                )

        # free-dim reduce of the three PSUM accumulators -> [C, 3] -> DRAM
        ob = cp.tile([C, 3], dt.float32)
        for k, acc in enumerate((ps_acc, in_acc, ts_acc)):
            nc.vector.tensor_reduce(
                out=ob[:, k : k + 1],
                in_=acc[:],
                axis=mybir.AxisListType.X,
                op=Alu.add,
            )
        nc.sync.dma_start(out=out_d[:], in_=ob[:])

    return nc


def _get_program():
    global _PROG
    if _PROG is None:
        _PROG = _build_program()
    return _PROG


def _shard_inputs(predict, target):
    x = np.ascontiguousarray(predict, dtype=np.float32).reshape(N, C, P, FTOT)
    x = x.astype(ml_dtypes.bfloat16)
    t = (
        np.ascontiguousarray(target)
        .reshape(N, P, FTOT)
        .astype(np.float32)
        .astype(ml_dtypes.bfloat16)
    )
    return [{"x": x[i], "t": t[i]} for i in range(N)]


def kernel(predict, target):
    from concourse.bass_utils import run_bass_kernel_spmd

    nc = _get_program()
    in_maps = _shard_inputs(predict, target)
    res = run_bass_kernel_spmd(nc, in_maps, list(range(NCORES)))
    stats = np.stack(
        [np.asarray(res.results[i]["out"], dtype=np.float32).reshape(C, 3) for i in range(NCORES)]
    )
    psum = stats[:, :, 0]
    inter = stats[:, :, 1]
    tsum = stats[:, :, 2]
    top = 2.0 * inter + 1.0
    bot = psum + tsum + 1.0
    per_class = np.mean(1.0 - top / bot, axis=0, dtype=np.float32)
    return np.float32(per_class.sum() / C)


# revision 22
# speedup vs baseline: 4.7113x; 1.0486x over previous
"""DiceLoss kernel for Trainium2 (8 NeuronCores, pure data parallel).

Problem: softmax over C=19 classes of predict [8, 19, 512, 512], one-hot of
target [8, 512, 512], then per-sample per-class sums
    psum[n,c]  = sum_pix softmax(x)[n,c,pix]
    inter[n,c] = sum_{pix: t=c} softmax(x)[n,c,pix]
    tsum[n,c]  = #{pix: t=c}
and dice = mean_c mean_n (1 - (2*inter+1)/(psum+tsum+1)).

Sharding: one sample per core (batch N=8 across 8 cores). Each core returns
[3*C] partial sums; the tiny final formula runs on host.

Device layout per core: x as [C, 128, 2048] bf16 (pixel-partition,
class-blocked free dim), processed in column chunks of F=512:
  - ScalarE: one Exp activation per chunk over all classes
  - DVE: pairwise-tree class sum -> denominator, reciprocal,
    then per class three fused ops:
      tensor_scalar(is_equal, accum)        -> one-hot mask + tsum column
      tensor_tensor_reduce(E*R, add-accum)  -> probs + psum column (chunk-chained)
      tensor_tensor_reduce(O*P, add-accum)  -> inter column (chunk-chained)
  - TensorE: final [128, 57] x ones matmul to reduce partitions.
Inputs are cast to bf16 on host (halves DMA bytes; fp32 accumulation of the
reductions keeps the loss error ~1e-4).
"""

import numpy as np
import ml_dtypes

N, C, H, W = 8, 19, 512, 512
PIX = H * W  # 262144
P = 128
FTOT = PIX // P  # 2048
F = 512
NCHUNK = FTOT // F
NCORES = 8

_PROG = None


def _build_program():
    from contextlib import ExitStack

    import concourse.bass as bass
    import concourse.tile as tile
    from concourse import mybir

    dt = mybir.dt
    Alu = mybir.AluOpType
    Act = mybir.ActivationFunctionType

    import bass_rust as _br

    class _TC(tile.TileContext):
        # This walrus build rejects instructions carrying more than ONE
        # sync-wait ("Too many sync wait commands", matching bass_rust's
        # inst_waits_full cap), but Tile's wait assignment occasionally
        # stacks 2-3 waits on one instruction and puts one wait per active
        # proc (12 here) on the tail drain. Hoist every excess wait onto an
        # injected same-engine single-wait Drain placed just before.

        def _split_excess_waits(self, insts):
            out = []
            for inst in insts:
                si = inst.sync_info
                if si is not None and len(si.on_wait) > 1:
                    moved = []
                    while len(si.on_wait) > 1:
                        moved.append(si.on_wait.pop())
                    for w in reversed(moved):
                        d = mybir.InstDrain(
                            name=self.nc.get_next_instruction_name(),
                            ins=[],
                            outs=[],
                            bass_is_fusable=False,
                        )
                        d.engine = inst.engine
                        d.sync_info = _br.SyncInfo(on_wait=[w], on_update=[])
                        self.nc.register_instruction(d)
                        out.append(d)
                out.append(inst)
            insts[:] = out

        def _lower_ordered_insts(self, ordered):
            for insts in ordered.values():
                self._split_excess_waits(insts)
            return super()._lower_ordered_insts(ordered)

        def _drain_and_barrier(self, tick_clock, wait_clock):
            from concourse.vector_clock import ScopedClock

            nc = self.nc
            drain_inst = nc.sync.drain()
            wait_clock.add_sem_waits(
                drain_inst.ins, ScopedClock({None: tick_clock.global_clock})
            )
            si = drain_inst.ins.sync_info
            moved = []
            while len(si.on_wait) > 1:
                moved.append(si.on_wait.pop())
            for w in moved:
                d2 = nc.sync.drain()
                d2.ins.sync_info = _br.SyncInfo(on_wait=[w], on_update=[])

            nc.all_engine_barrier()
            assert self.sems is not None
            popped = nc._tile_sem_poison_stack.pop()
            assert popped is self._sem_poison
            nc.clear_and_free_semaphores(list(self.sems.allocated().values()))
            nc.all_engine_barrier()

    nc = bass.Bass(
        "TRN2", target_bir_lowering=False, debug=False, num_devices=NCORES
    )
    x_d = nc.dram_tensor("x", [C, P, FTOT], dt.bfloat16, kind="ExternalInput").ap()
    t_d = nc.dram_tensor("t", [P, FTOT], dt.bfloat16, kind="ExternalInput").ap()
    out_d = nc.dram_tensor("out", [C, 3], dt.float32, kind="ExternalOutput").ap()

    with nc.allow_low_precision("bf16 softmax-stat kernel"), \
            _TC(nc) as tc, ExitStack() as ctx:
        # DMA-written tiles get one slot per chunk: a DMACopy may carry at
        # most ONE sync-wait on TRN2, so slot reuse (which adds WAR/WAW
        # waits onto the DMA) must be avoided for them.
        xp = ctx.enter_context(tc.tile_pool(name="xp", bufs=NCHUNK))
        ep = ctx.enter_context(tc.tile_pool(name="ep", bufs=2))
        tp = ctx.enter_context(tc.tile_pool(name="tp", bufs=NCHUNK))
        dp = ctx.enter_context(tc.tile_pool(name="dp", bufs=2))
        sp = ctx.enter_context(tc.tile_pool(name="sp", bufs=3))
        cp = ctx.enter_context(tc.tile_pool(name="cp", bufs=1))
        pp = ctx.enter_context(tc.tile_pool(name="pp", bufs=1, space="PSUM"))

        # per-class one-hot lhsT columns: block c is a [P, C] matrix whose
        # column c is all-ones -> matmul with rhs [P, F] lands the
        # pixel-partition sums of rhs on PSUM partition c.
        cols = cp.tile([P, C * C], dt.bfloat16)
        nc.vector.memset(cols[:], 0.0)
        for c in range(C):
            nc.vector.memset(cols[:, c * C + c : c * C + c + 1], 1.0)

        ps_acc = pp.tile([C, F], dt.float32)
        in_acc = pp.tile([C, F], dt.float32)
        ts_acc = pp.tile([C, F], dt.float32)

        for j in range(NCHUNK):
            xt = xp.tile([P, C * F], dt.bfloat16, tag="x")
            nc.sync.dma_start(
                out=xt[:].rearrange("p (c f) -> p c f", c=C),
                in_=x_d[:, :, j * F : (j + 1) * F].rearrange("c p f -> p c f"),
            )
            tt = tp.tile([P, F], dt.bfloat16, tag="t")
            nc.scalar.dma_start(out=tt[:], in_=t_d[:, j * F : (j + 1) * F])

            et = ep.tile([P, C * F], dt.bfloat16, tag="e")
            nc.scalar.activation(et[:], xt[:], Act.Exp)
            ev = et[:].rearrange("p (c f) -> p c f", c=C)

            # denominator: tree-sum the 19 class blocks (bf16 2x mode),
            # last add in fp32 for reciprocal_approx_fast
            s9 = sp.tile([P, 9 * F], dt.bfloat16, tag="s9")
            s9v = s9[:].rearrange("p (c f) -> p c f", c=9)
            nc.vector.tensor_tensor(
                s9v[:, :, :], ev[:, 0:18:2, :], ev[:, 1:19:2, :], Alu.add
            )
            s4 = sp.tile([P, 4 * F], dt.bfloat16, tag="s4")
            s4v = s4[:].rearrange("p (c f) -> p c f", c=4)
            nc.vector.tensor_tensor(
                s4v[:, :, :], s9v[:, 0:8:2, :], s9v[:, 1:9:2, :], Alu.add
            )
            s2 = sp.tile([P, 2 * F], dt.bfloat16, tag="s2")
            s2v = s2[:].rearrange("p (c f) -> p c f", c=2)
            nc.vector.tensor_tensor(
                s2v[:, :, :], s4v[:, 0:4:2, :], s4v[:, 1:4:2, :], Alu.add
            )
            s1 = sp.tile([P, F], dt.bfloat16, tag="s1")
            nc.vector.tensor_tensor(s1[:], s2v[:, 0, :], s2v[:, 1, :], Alu.add)
            d0 = sp.tile([P, F], dt.bfloat16, tag="d0")
            nc.vector.tensor_tensor(d0[:], s1[:], s9v[:, 8, :], Alu.add)
            dd = sp.tile([P, F], dt.bfloat16, tag="dd")
            nc.vector.tensor_tensor(dd[:], d0[:], ev[:, 18, :], Alu.add)
            rt = dp.tile([P, F], dt.bfloat16, tag="r")
            nc.vector.reciprocal(rt[:], dd[:])

            for c in range(C):
                first = j == 0 and c == 0
                last = j == NCHUNK - 1 and c == C - 1
                lhs = cols[:, c * C : (c + 1) * C]
                o = sp.tile([P, F], dt.bfloat16, tag="o", bufs=8)
                nc.vector.tensor_scalar(o[:], tt[:], float(c), None, Alu.is_equal)
                pc = sp.tile([P, F], dt.bfloat16, tag="pc", bufs=8)
                nc.vector.tensor_tensor(pc[:], ev[:, c, :], rt[:], Alu.mult)
                oc = sp.tile([P, F], dt.bfloat16, tag="oc", bufs=8)
                nc.vector.tensor_tensor(oc[:], o[:], pc[:], Alu.mult)
                nc.tensor.matmul(
                    ps_acc[:], lhsT=lhs, rhs=pc[:], start=first, stop=last
                )
                nc.tensor.matmul(
                    in_acc[:], lhsT=lhs, rhs=oc[:], start=first, stop=last
                )
                nc.tensor.matmul(
                    ts_acc[:], lhsT=lhs, rhs=o[:], start=first, stop=last
                )

        # free-dim reduce of the three PSUM accumulators -> [C, 3] -> DRAM
        ob = cp.tile([C, 3], dt.float32)
        for k, acc in enumerate((ps_acc, in_acc, ts_acc)):
            nc.vector.tensor_reduce(
                out=ob[:, k : k + 1],
                in_=acc[:],
                axis=mybir.AxisListType.X,
                op=Alu.add,
            )
        nc.sync.dma_start(out=out_d[:], in_=ob[:])

    return nc


def _get_program():
    global _PROG
    if _PROG is None:
        _PROG = _build_program()
    return _PROG


def _shard_inputs(predict, target):
    x = np.ascontiguousarray(predict, dtype=np.float32).reshape(N, C, P, FTOT)
    x = x.astype(ml_dtypes.bfloat16)
    t = (
        np.ascontiguousarray(target)
        .reshape(N, P, FTOT)
        .astype(np.float32)
        .astype(ml_dtypes.bfloat16)
    )
    return [{"x": x[i], "t": t[i]} for i in range(N)]


def kernel(predict, target):
    from concourse.bass_utils import run_bass_kernel_spmd

    nc = _get_program()
    in_maps = _shard_inputs(predict, target)
    res = run_bass_kernel_spmd(nc, in_maps, list(range(NCORES)))
    stats = np.stack(
        [np.asarray(res.results[i]["out"], dtype=np.float32).reshape(C, 3) for i in range(NCORES)]
    )
    psum = stats[:, :, 0]
    inter = stats[:, :, 1]
    tsum = stats[:, :, 2]
    top = 2.0 * inter + 1.0
    bot = psum + tsum + 1.0
    per_class = np.mean(1.0 - top / bot, axis=0, dtype=np.float32)
    return np.float32(per_class.sum() / C)


# revision 23
# speedup vs baseline: 4.8663x; 1.0329x over previous
"""DiceLoss kernel for Trainium2 (8 NeuronCores, pure data parallel).

Problem: softmax over C=19 classes of predict [8, 19, 512, 512], one-hot of
target [8, 512, 512], then per-sample per-class sums
    psum[n,c]  = sum_pix softmax(x)[n,c,pix]
    inter[n,c] = sum_{pix: t=c} softmax(x)[n,c,pix]
    tsum[n,c]  = #{pix: t=c}
and dice = mean_c mean_n (1 - (2*inter+1)/(psum+tsum+1)).

Sharding: one sample per core (batch N=8 across 8 cores). Each core returns
[3*C] partial sums; the tiny final formula runs on host.

Device layout per core: x as [C, 128, 2048] bf16 (pixel-partition,
class-blocked free dim), processed in column chunks of F=512:
  - ScalarE: one Exp activation per chunk over all classes
  - DVE: pairwise-tree class sum -> denominator, reciprocal,
    then per class three fused ops:
      tensor_scalar(is_equal, accum)        -> one-hot mask + tsum column
      tensor_tensor_reduce(E*R, add-accum)  -> probs + psum column (chunk-chained)
      tensor_tensor_reduce(O*P, add-accum)  -> inter column (chunk-chained)
  - TensorE: final [128, 57] x ones matmul to reduce partitions.
Inputs are cast to bf16 on host (halves DMA bytes; fp32 accumulation of the
reductions keeps the loss error ~1e-4).
"""

import numpy as np
import ml_dtypes

N, C, H, W = 8, 19, 512, 512
PIX = H * W  # 262144
P = 128
FTOT = PIX // P  # 2048
F = 512
NCHUNK = FTOT // F
NCORES = 8

_PROG = None


def _build_program():
    from contextlib import ExitStack

    import concourse.bass as bass
    import concourse.tile as tile
    from concourse import mybir

    dt = mybir.dt
    Alu = mybir.AluOpType
    Act = mybir.ActivationFunctionType

    import bass_rust as _br

    class _TC(tile.TileContext):
        # This walrus build rejects instructions carrying more than ONE
        # sync-wait ("Too many sync wait commands", matching bass_rust's
        # inst_waits_full cap), but Tile's wait assignment occasionally
        # stacks 2-3 waits on one instruction and puts one wait per active
        # proc (12 here) on the tail drain. Hoist every excess wait onto an
        # injected same-engine single-wait Drain placed just before.

        def _split_excess_waits(self, insts):
            out = []
            for inst in insts:
                si = inst.sync_info
                if si is not None and len(si.on_wait) > 1:
                    moved = []
                    while len(si.on_wait) > 1:
                        moved.append(si.on_wait.pop())
                    for w in reversed(moved):
                        d = mybir.InstDrain(
                            name=self.nc.get_next_instruction_name(),
                            ins=[],
                            outs=[],
                            bass_is_fusable=False,
                        )
                        d.engine = inst.engine
                        d.sync_info = _br.SyncInfo(on_wait=[w], on_update=[])
                        self.nc.register_instruction(d)
                        out.append(d)
                out.append(inst)
            insts[:] = out

        def _lower_ordered_insts(self, ordered):
            for insts in ordered.values():
                self._split_excess_waits(insts)
            return super()._lower_ordered_insts(ordered)

        def _drain_and_barrier(self, tick_clock, wait_clock):
            from concourse.vector_clock import ScopedClock

            nc = self.nc
            drain_inst = nc.sync.drain()
            wait_clock.add_sem_waits(
                drain_inst.ins, ScopedClock({None: tick_clock.global_clock})
            )
            si = drain_inst.ins.sync_info
            moved = []
            while len(si.on_wait) > 1:
                moved.append(si.on_wait.pop())
            for w in moved:
                d2 = nc.sync.drain()
                d2.ins.sync_info = _br.SyncInfo(on_wait=[w], on_update=[])

            nc.all_engine_barrier()
            assert self.sems is not None
            popped = nc._tile_sem_poison_stack.pop()
            assert popped is self._sem_poison
            nc.clear_and_free_semaphores(list(self.sems.allocated().values()))
            nc.all_engine_barrier()

    nc = bass.Bass(
        "TRN2", target_bir_lowering=False, debug=False, num_devices=NCORES
    )
    x_d = nc.dram_tensor("x", [C, P, FTOT], dt.bfloat16, kind="ExternalInput").ap()
    t_d = nc.dram_tensor("t", [P, FTOT], dt.bfloat16, kind="ExternalInput").ap()
    out_d = nc.dram_tensor("out", [C, 3], dt.float32, kind="ExternalOutput").ap()

    with nc.allow_low_precision("bf16 softmax-stat kernel"), \
            _TC(nc) as tc, ExitStack() as ctx:
        # DMA-written tiles get one slot per chunk: a DMACopy may carry at
        # most ONE sync-wait on TRN2, so slot reuse (which adds WAR/WAW
        # waits onto the DMA) must be avoided for them.
        xp = ctx.enter_context(tc.tile_pool(name="xp", bufs=3))
        ep = ctx.enter_context(tc.tile_pool(name="ep", bufs=2))
        tp = ctx.enter_context(tc.tile_pool(name="tp", bufs=NCHUNK))
        dp = ctx.enter_context(tc.tile_pool(name="dp", bufs=2))
        sp = ctx.enter_context(tc.tile_pool(name="sp", bufs=3))
        cp = ctx.enter_context(tc.tile_pool(name="cp", bufs=1))
        pp = ctx.enter_context(tc.tile_pool(name="pp", bufs=1, space="PSUM"))

        # per-class one-hot lhsT columns: block c is a [P, C] matrix whose
        # column c is all-ones -> matmul with rhs [P, F] lands the
        # pixel-partition sums of rhs on PSUM partition c.
        cols = cp.tile([P, C * C], dt.bfloat16)
        nc.vector.memset(cols[:], 0.0)
        for c in range(C):
            nc.vector.memset(cols[:, c * C + c : c * C + c + 1], 1.0)

        ps_acc = pp.tile([C, F], dt.float32)
        in_acc = pp.tile([C, F], dt.float32)
        ts_acc = pp.tile([C, F], dt.float32)

        for j in range(NCHUNK):
            xt = xp.tile([P, C * F], dt.bfloat16, tag="x")
            xv = xt[:].rearrange("p (c f) -> p c f", c=C)
            et = ep.tile([P, C * F], dt.bfloat16, tag="e")
            ev = et[:].rearrange("p (c f) -> p c f", c=C)
            CSPLIT = 10
            for c0, c1 in ((0, CSPLIT), (CSPLIT, C)):
                nc.sync.dma_start(
                    out=xv[:, c0:c1, :],
                    in_=x_d[c0:c1, :, j * F : (j + 1) * F].rearrange(
                        "c p f -> p c f"
                    ),
                )
                nc.scalar.activation(
                    et[:, c0 * F : c1 * F], xt[:, c0 * F : c1 * F], Act.Exp
                )
            tt = tp.tile([P, F], dt.bfloat16, tag="t")
            nc.scalar.dma_start(out=tt[:], in_=t_d[:, j * F : (j + 1) * F])

            # denominator: tree-sum the 19 class blocks (bf16 2x mode),
            # last add in fp32 for reciprocal_approx_fast
            s9 = sp.tile([P, 9 * F], dt.bfloat16, tag="s9", bufs=1)
            s9v = s9[:].rearrange("p (c f) -> p c f", c=9)
            nc.vector.tensor_tensor(
                s9v[:, :, :], ev[:, 0:18:2, :], ev[:, 1:19:2, :], Alu.add
            )
            s4 = sp.tile([P, 4 * F], dt.bfloat16, tag="s4", bufs=1)
            s4v = s4[:].rearrange("p (c f) -> p c f", c=4)
            nc.vector.tensor_tensor(
                s4v[:, :, :], s9v[:, 0:8:2, :], s9v[:, 1:9:2, :], Alu.add
            )
            s2 = sp.tile([P, 2 * F], dt.bfloat16, tag="s2", bufs=1)
            s2v = s2[:].rearrange("p (c f) -> p c f", c=2)
            nc.vector.tensor_tensor(
                s2v[:, :, :], s4v[:, 0:4:2, :], s4v[:, 1:4:2, :], Alu.add
            )
            s1 = sp.tile([P, F], dt.bfloat16, tag="s1", bufs=1)
            nc.vector.tensor_tensor(s1[:], s2v[:, 0, :], s2v[:, 1, :], Alu.add)
            d0 = sp.tile([P, F], dt.bfloat16, tag="d0", bufs=1)
            nc.vector.tensor_tensor(d0[:], s1[:], s9v[:, 8, :], Alu.add)
            dd = sp.tile([P, F], dt.bfloat16, tag="dd", bufs=1)
            nc.vector.tensor_tensor(dd[:], d0[:], ev[:, 18, :], Alu.add)
            rt = dp.tile([P, F], dt.bfloat16, tag="r")
            nc.vector.reciprocal(rt[:], dd[:])

            for c in range(C):
                first = j == 0 and c == 0
                last = j == NCHUNK - 1 and c == C - 1
                lhs = cols[:, c * C : (c + 1) * C]
                o = sp.tile([P, F], dt.bfloat16, tag="o", bufs=12)
                nc.vector.tensor_scalar(o[:], tt[:], float(c), None, Alu.is_equal)
                pc = sp.tile([P, F], dt.bfloat16, tag="pc", bufs=12)
                nc.vector.tensor_tensor(pc[:], ev[:, c, :], rt[:], Alu.mult)
                oc = sp.tile([P, F], dt.bfloat16, tag="oc", bufs=12)
                nc.vector.tensor_tensor(oc[:], o[:], pc[:], Alu.mult)
                nc.tensor.matmul(
                    ps_acc[:], lhsT=lhs, rhs=pc[:], start=first, stop=last
                )
                nc.tensor.matmul(
                    in_acc[:], lhsT=lhs, rhs=oc[:], start=first, stop=last
                )
                nc.tensor.matmul(
                    ts_acc[:], lhsT=lhs, rhs=o[:], start=first, stop=last
                )

        # free-dim reduce of the three PSUM accumulators -> [C, 3] -> DRAM
        ob = cp.tile([C, 3], dt.float32)
        for k, acc in enumerate((ps_acc, in_acc, ts_acc)):
            nc.vector.tensor_reduce(
                out=ob[:, k : k + 1],
                in_=acc[:],
                axis=mybir.AxisListType.X,
                op=Alu.add,
            )
        nc.sync.dma_start(out=out_d[:], in_=ob[:])

    return nc


def _get_program():
    global _PROG
    if _PROG is None:
        _PROG = _build_program()
    return _PROG


def _shard_inputs(predict, target):
    x = np.ascontiguousarray(predict, dtype=np.float32).reshape(N, C, P, FTOT)
    x = x.astype(ml_dtypes.bfloat16)
    t = (
        np.ascontiguousarray(target)
        .reshape(N, P, FTOT)
        .astype(np.float32)
        .astype(ml_dtypes.bfloat16)
    )
    return [{"x": x[i], "t": t[i]} for i in range(N)]


def kernel(predict, target):
    from concourse.bass_utils import run_bass_kernel_spmd

    nc = _get_program()
    in_maps = _shard_inputs(predict, target)
    res = run_bass_kernel_spmd(nc, in_maps, list(range(NCORES)))
    stats = np.stack(
        [np.asarray(res.results[i]["out"], dtype=np.float32).reshape(C, 3) for i in range(NCORES)]
    )
    psum = stats[:, :, 0]
    inter = stats[:, :, 1]
    tsum = stats[:, :, 2]
    top = 2.0 * inter + 1.0
    bot = psum + tsum + 1.0
    per_class = np.mean(1.0 - top / bot, axis=0, dtype=np.float32)
    return np.float32(per_class.sum() / C)
